# revision 23
# baseline (speedup 1.0000x reference)
"""Cumulative-probability head on 8 Trainium2 NeuronCores.

out[b, j] = sum_{i<=j} relu(x @ W_h^T + b_h)[b, i] + (x @ W_base^T + b_base)[b]

Data-parallel: x is sharded along batch (1024 rows per core); the small
weights are replicated. Inputs are cast to fp8 e4m3 on the host (the
2e-2 rel-err budget dwarfs the ~4e-3 this costs) so the matmul runs in
DoubleRow mode: 2 fp8 weights per PE cell, K=256 contracted per pass —
half the PE stream cycles of fp32r, and a quarter of the input DMA
bytes. Per core:

  1. xt4: per-batch-tile blocks [128, 16, 128] fp8 (contiguous 256 KB
     DMA each). wt3: [128, 18, 528] fp8 (528 = 512 hazard cols + base
     col + 15 pad, keeping the DoubleRow k-pair stride 1056 B a
     multiple of 16); subtile 16 carries 16*bias in partition 0,
     subtile 17 is zero. A memset tile of 0.0625 (exact in e4m3)
     supplies the ones operand: k-pair 8 = ones x (16*bias, 0)
     contributes exactly the bias row, folding the bias add into the
     matmul. DMA FIFO order per ring is need-order: all w chunks ahead
     of x blocks 1..7, since b-tile 0 consumes one w chunk per ~240 ns
     and its completion gates the whole epilogue pipeline.
  2. The PE HAM clock-gate needs ~3.4 us of sustained busy to lift the
     1.2 GHz cold throttle, and a mid-kernel idle gap >3.4 us drops it
     back. ~30 dummy matmuls on a zeroed tile burn the warmup during
     the input-DMA window; each b-tile's group opens with the bias
     pair (no x dependency, schedulable early) plus a few dummy
     matmuls on the next PSUM instance to bridge input-DMA jitter
     without going idle.
  3. Per b-tile: 9 DoubleRow accumulation steps into one two-bank PSUM
     tile [128, 2, 512] (cols 0..263 of each bank); bufs=4 -> 4
     b-tiles in flight across all 8 banks.
  4. Epilogue per b-tile: one ScalarE ReLU over a strided 2-bank PSUM
     read (bf16 out), then the DVE cumsum (tensor_tensor_scan) seeded
     with the base-hazard column read straight from PSUM. Output DMA
     in bf16 on the Sync ring (ScalarE stays on ReLUs).
"""

import ml_dtypes
import numpy as np

import concourse.bass as bass
import concourse.tile as tile
from concourse import bacc, mybir
from concourse.bass_utils import run_bass_kernel_spmd

B, D, T = 8192, 2048, 512
NCORES = 8
BLOC = B // NCORES            # 1024 rows per core
NB = BLOC // 128              # 8 batch tiles per core
NKS = D // 128                # 16 data contraction subtiles
NWS = NKS + 2                 # +bias +zeros subtiles in w
NU = NKS // 2 + 1             # 9 DoubleRow k-pairs (bias + 8 data)
TPW = 528                     # padded output width (512 hazard + base + 15 pad)
NA = TPW // 2                 # 264 cols per PSUM bank
NBC = 250                     # bank-B streamed cols (248 hazards + base + 1)
NWARM = 20                    # HAM warmup matmuls
NFILL = 6                     # stall-filler dummies before early tiles
ONES = 0.0625                 # exact in e4m3; bias row is scaled by 1/ONES

F32 = mybir.dt.float32
BF16 = mybir.dt.bfloat16
F8 = mybir.dt.float8e4
DR = mybir.MatmulPerfMode.DoubleRow
NP_F8 = ml_dtypes.float8_e4m3


def _build_program():
    nc = bacc.Bacc("TRN2", target_bir_lowering=False, debug=False)

    xt_d = nc.dram_tensor("xt", [NB, 128, NKS * 128], F8, kind="ExternalInput")
    wt_d = nc.dram_tensor("wt", [128, NWS, TPW], F8, kind="ExternalInput")
    out_d = nc.dram_tensor("out", [BLOC, T], BF16, kind="ExternalOutput")

    with tile.TileContext(nc) as tc:
        with (
            tc.tile_pool(name="consts", bufs=1) as consts,
            tc.tile_pool(name="wt", bufs=1) as wtp,
            tc.tile_pool(name="xt", bufs=1) as xtp,
            tc.tile_pool(name="haz", bufs=6) as hazp,
            tc.tile_pool(name="outp", bufs=6) as outp,
            tc.tile_pool(name="ps_mm", bufs=4, space="PSUM") as ps_mm,
        ):
            zeros = consts.tile([128, T], BF16)
            nc.vector.memset(zeros, 0.0)
            warm_f8 = consts.tile([128, 128], F8)
            nc.vector.memset(warm_f8, 0.0)
            ones_f8 = consts.tile([128, 2, 128], F8)
            nc.vector.memset(ones_f8, ONES)

            wt_sb = [
                wtp.tile([128, 2, TPW], F8, tag=f"wt{u}", name=f"wt{u}")
                for u in range(NU)
            ]
            # The bias chunk is zero except partition 0 of subtile 16:
            # memset it and DMA just the 528-byte bias row.
            nc.vector.memset(wt_sb[NU - 1], 0.0)
            xb0a = xtp.tile([128, 8, 128], F8, tag="xb0a")
            xb0b = xtp.tile([128, 8, 128], F8, tag="xb0b")
            xb = [
                xtp.tile([128, NKS, 128], F8, tag=f"xb{b}", name=f"xb{b}")
                for b in range(1, NB)
            ]

            def load_w(ring, u):
                ring.dma_start(
                    out=wt_sb[u][:], in_=wt_d[:, 2 * u : 2 * u + 2, :]
                )

            def load_x(ring, b):
                ring.dma_start(out=xb[b - 1][:], in_=xt_d[b, :, :])

            def x_pair(b, u):
                j0 = 2 * u
                if b == 0:
                    return (
                        xb0a[:, j0 : j0 + 2, :]
                        if j0 < 8
                        else xb0b[:, j0 - 8 : j0 - 6, :]
                    )
                return xb[b - 1][:, j0 : j0 + 2, :]

            # Need-ordered FIFO per ring: w pair 0 and both halves of x
            # block 0 gate the first matmuls; all remaining w chunks
            # (which gate b-tile 0's completion) ahead of x blocks.
            # The bias subtile (66 KB, full 128 partitions — tiny
            # single-partition DMAs route through a slow path) leads
            # its ring: the bias pair opens every accumulation group,
            # so it gates every tile.
            nc.gpsimd.dma_start(
                out=wt_sb[NU - 1][:, 0:1, :], in_=wt_d[:, NKS : NKS + 1, :]
            )
            load_w(nc.sync, 0)
            load_w(nc.scalar, 1)
            nc.sync.dma_start(out=xb0a[:], in_=xt_d[0, :, 0 : 8 * 128])
            nc.scalar.dma_start(out=xb0b[:], in_=xt_d[0, :, 8 * 128 :])
            load_w(nc.gpsimd, 2)
            load_w(nc.sync, 3)
            load_w(nc.scalar, 4)
            load_w(nc.gpsimd, 5)
            load_w(nc.sync, 6)
            load_w(nc.scalar, 7)
            load_x(nc.gpsimd, 1)
            load_x(nc.scalar, 2)
            load_x(nc.sync, 3)
            load_x(nc.gpsimd, 4)
            load_x(nc.scalar, 5)
            load_x(nc.gpsimd, 6)
            load_x(nc.scalar, 7)

            # HAM warmup: dummies depend only on the memset tile, so
            # they run during the input-DMA window and lift the PE
            # clock gate to 8/8 before the first real matmul.
            ps_warm = ps_mm.tile([128, 2, 512], F32, tag="pAB", name="ps_warm")
            for i in range(NWARM):
                nc.tensor.matmul(
                    ps_warm[:, 0, 0:128], warm_f8, warm_f8,
                    start=True, stop=True, skip_group_check=True,
                )

            for b in range(NB):
                pAB = (
                    ps_warm if b == 0
                    else ps_mm.tile([128, 2, 512], F32, tag="pAB", name="pAB")
                )
                if 1 <= b <= 4:
                    # Bridge input-DMA jitter: a burst of dummies on
                    # this tile's (about-to-be-cleared) PSUM instance
                    # keeps the PE from idling into a HAM re-throttle
                    # while this b-tile's x block is still in flight.
                    for i in range(NFILL):
                        nc.tensor.matmul(
                            pAB[:, 0, 0:128], warm_f8, warm_f8,
                            start=True, stop=True, skip_group_check=True,
                        )
                for u in range(NU):
                    if u == 0:
                        # Bias pair first: ones x (16*bias, 0) — no x
                        # dependency, so the scheduler can hoist it
                        # into DMA-wait gaps.
                        lhsT = ones_f8[:]
                        w = wt_sb[NU - 1]
                    else:
                        lhsT = x_pair(b, u - 1)
                        w = wt_sb[u - 1]
                    nc.tensor.matmul(
                        pAB[:, 0, 0:NA], lhsT, w[:, :, 0:NA],
                        start=(u == 0), stop=(u == NU - 1), perf_mode=DR,
                        skip_group_check=True,
                    )
                    nc.tensor.matmul(
                        pAB[:, 1, 0:NBC], lhsT, w[:, :, NA : NA + NBC],
                        start=(u == 0), stop=(u == NU - 1), perf_mode=DR,
                        skip_group_check=True,
                    )

                # PSUM flat layout: cols 0..511 hazards, 512 base; the
                # relu'd base/pad cols 512..527 are junk the scan skips.
                haz = hazp.tile([128, TPW], BF16, tag="haz")
                nc.scalar.activation(
                    out=haz, in_=pAB[:, :, 0:NA],
                    func=mybir.ActivationFunctionType.Relu,
                )
                cum = outp.tile([128, T], BF16)
                nc.vector.tensor_tensor_scan(
                    out=cum,
                    data0=haz[:, 0:T],
                    data1=zeros,
                    initial=pAB[:, 1, T - NA : T - NA + 1],
                    op0=mybir.AluOpType.add,
                    op1=mybir.AluOpType.bypass,
                )
                if b >= NB - 2:
                    # Tail tiles: split the write across two rings so
                    # the final transfer halves.
                    nc.sync.dma_start(
                        out=out_d[128 * b : 128 * b + 64, :], in_=cum[0:64, :]
                    )
                    nc.scalar.dma_start(
                        out=out_d[128 * b + 64 : 128 * (b + 1), :],
                        in_=cum[64:128, :],
                    )
                else:
                    nc.sync.dma_start(
                        out=out_d[128 * b : 128 * (b + 1), :], in_=cum
                    )

    nc.compile()
    return nc


def _prep_in_maps(x, W_hazard, b_hazard, W_base, b_base):
    x = np.asarray(x, dtype=np.float32)
    W_aug = np.zeros((TPW, D), np.float32)
    W_aug[0:T] = np.asarray(W_hazard, np.float32)
    W_aug[T] = np.asarray(W_base, np.float32)[0]
    wt3 = np.zeros((128, NWS, TPW), NP_F8)
    wt3[:, 0:NKS, :] = (
        W_aug.T.astype(NP_F8).reshape(NKS, 128, TPW).transpose(1, 0, 2)
    )
    bias_row = np.zeros(TPW, np.float32)
    bias_row[0:T] = np.asarray(b_hazard, np.float32)
    bias_row[T] = np.asarray(b_base, np.float32)[0]
    wt3[0, NKS, :] = (bias_row / ONES).astype(NP_F8)
    wt3 = np.ascontiguousarray(wt3)

    x8 = x.astype(NP_F8)
    in_maps = []
    for i in range(NCORES):
        shard = x8[BLOC * i : BLOC * (i + 1)]  # [1024, 2048]
        # xt4[b, p, j*128+c] = shard[128b+c, 128j+p]
        xt4 = np.ascontiguousarray(
            shard.reshape(NB, 128, NKS, 128)  # [b, c, j, p]
            .transpose(0, 3, 2, 1)            # [b, p, j, c]
            .reshape(NB, 128, NKS * 128)
        )
        in_maps.append({"xt": xt4, "wt": wt3})
    return in_maps


_NC_CACHE = None


def kernel(x, W_hazard, b_hazard, W_base, b_base):
    global _NC_CACHE
    if _NC_CACHE is None:
        _NC_CACHE = _build_program()
    nc = _NC_CACHE

    in_maps = _prep_in_maps(x, W_hazard, b_hazard, W_base, b_base)
    res = run_bass_kernel_spmd(nc, in_maps, list(range(NCORES)))
    return np.concatenate(
        [res.results[i]["out"].astype(np.float32) for i in range(NCORES)], axis=0
    )


# revision 25
# speedup vs baseline: 1.1411x; 1.1411x over previous
"""Cumulative-probability head on 8 Trainium2 NeuronCores.

out[b, j] = sum_{i<=j} relu(x @ W_h^T + b_h)[b, i] + (x @ W_base^T + b_base)[b]

Data-parallel: x is sharded along batch (1024 rows per core); the small
weights are replicated. Inputs are cast to fp8 e4m3 on the host (the
2e-2 rel-err budget dwarfs the ~4e-3 this costs) so the matmul runs in
DoubleRow mode: 2 fp8 weights per PE cell, K=256 contracted per pass —
half the PE stream cycles of fp32r, and a quarter of the input DMA
bytes. Per core:

  1. xt4: per-batch-tile blocks [128, 16, 128] fp8 (contiguous 256 KB
     DMA each). wt3: [128, 18, 528] fp8 (528 = 512 hazard cols + base
     col + 15 pad, keeping the DoubleRow k-pair stride 1056 B a
     multiple of 16); subtile 16 carries 16*bias in partition 0,
     subtile 17 is zero. A memset tile of 0.0625 (exact in e4m3)
     supplies the ones operand: k-pair 8 = ones x (16*bias, 0)
     contributes exactly the bias row, folding the bias add into the
     matmul. DMA FIFO order per ring is need-order: all w chunks ahead
     of x blocks 1..7, since b-tile 0 consumes one w chunk per ~240 ns
     and its completion gates the whole epilogue pipeline.
  2. The PE HAM clock-gate needs ~3.4 us of sustained busy to lift the
     1.2 GHz cold throttle, and a mid-kernel idle gap >3.4 us drops it
     back. ~30 dummy matmuls on a zeroed tile burn the warmup during
     the input-DMA window; each b-tile's group opens with the bias
     pair (no x dependency, schedulable early) plus a few dummy
     matmuls on the next PSUM instance to bridge input-DMA jitter
     without going idle.
  3. Per b-tile: 9 DoubleRow accumulation steps into one two-bank PSUM
     tile [128, 2, 512] (cols 0..263 of each bank); bufs=4 -> 4
     b-tiles in flight across all 8 banks.
  4. Epilogue per b-tile: one ScalarE ReLU over a strided 2-bank PSUM
     read (bf16 out), then the DVE cumsum (tensor_tensor_scan) seeded
     with the base-hazard column read straight from PSUM. Output DMA
     in bf16 on the Sync ring (ScalarE stays on ReLUs).
"""

import ml_dtypes
import numpy as np

import concourse.bass as bass
import concourse.tile as tile
from concourse import bacc, mybir
from concourse.bass_utils import run_bass_kernel_spmd

B, D, T = 8192, 2048, 512
NCORES = 8
BLOC = B // NCORES            # 1024 rows per core
NB = BLOC // 128              # 8 batch tiles per core
NKS = D // 128                # 16 data contraction subtiles
NWS = NKS + 2                 # +bias +zeros subtiles in w
NU = NKS // 2 + 1             # 9 DoubleRow k-pairs (bias + 8 data)
TPW = 528                     # padded output width (512 hazard + base + 15 pad)
NA = TPW // 2                 # 264 cols per PSUM bank
NBC = 250                     # bank-B streamed cols (248 hazards + base + 1)
NWARM = 20                    # HAM warmup matmuls
NFILL = 6                     # stall-filler dummies before early tiles
ONES = 0.0625                 # exact in e4m3; bias row is scaled by 1/ONES

F32 = mybir.dt.float32
BF16 = mybir.dt.bfloat16
F8 = mybir.dt.float8e4
DR = mybir.MatmulPerfMode.DoubleRow
NP_F8 = ml_dtypes.float8_e4m3


def _build_program():
    nc = bacc.Bacc("TRN2", target_bir_lowering=False, debug=False)

    xt_d = nc.dram_tensor("xt", [NB, 128, NKS * 128], F8, kind="ExternalInput")
    wt_d = nc.dram_tensor("wt", [128, NWS, TPW], F8, kind="ExternalInput")
    out_d = nc.dram_tensor("out", [BLOC, T], BF16, kind="ExternalOutput")

    with tile.TileContext(nc) as tc:
        with (
            tc.tile_pool(name="consts", bufs=1) as consts,
            tc.tile_pool(name="wt", bufs=1) as wtp,
            tc.tile_pool(name="xt", bufs=1) as xtp,
            tc.tile_pool(name="haz", bufs=6) as hazp,
            tc.tile_pool(name="outp", bufs=6) as outp,
            tc.tile_pool(name="ps_mm", bufs=4, space="PSUM") as ps_mm,
        ):
            zeros = consts.tile([128, T], BF16)
            nc.vector.memset(zeros, 0.0)
            warm_f8 = consts.tile([128, 128], F8)
            nc.vector.memset(warm_f8, 0.0)
            ones_f8 = consts.tile([128, 2, 128], F8)
            nc.vector.memset(ones_f8, ONES)

            wt_sb = [
                wtp.tile([128, 2, TPW], F8, tag=f"wt{u}", name=f"wt{u}")
                for u in range(NU)
            ]

            xb0a = xtp.tile([128, 8, 128], F8, tag="xb0a")
            xb0b = xtp.tile([128, 8, 128], F8, tag="xb0b")
            xb = [
                xtp.tile([128, NKS, 128], F8, tag=f"xb{b}", name=f"xb{b}")
                for b in range(1, NB)
            ]

            def load_w(ring, u):
                ring.dma_start(
                    out=wt_sb[u][:], in_=wt_d[:, 2 * u : 2 * u + 2, :]
                )

            def load_x(ring, b):
                ring.dma_start(out=xb[b - 1][:], in_=xt_d[b, :, :])

            def x_pair(b, u):
                j0 = 2 * u
                if b == 0:
                    return (
                        xb0a[:, j0 : j0 + 2, :]
                        if j0 < 8
                        else xb0b[:, j0 - 8 : j0 - 6, :]
                    )
                return xb[b - 1][:, j0 : j0 + 2, :]

            # Need-ordered FIFO per ring: w pair 0 and both halves of x
            # block 0 gate the first matmuls; all remaining w chunks
            # (which gate b-tile 0's completion) ahead of x blocks.
            # w8 (the bias chunk) leads its ring as a plain dependency-
            # free 132 KB transfer: the bias pair opens every
            # accumulation group, so it gates every tile, and giving
            # it any producer dependency (e.g. a memset of its zero
            # half) makes the scheduler demote its issue slot.
            load_w(nc.gpsimd, 8)
            load_w(nc.sync, 0)
            load_w(nc.scalar, 1)
            nc.sync.dma_start(out=xb0a[:], in_=xt_d[0, :, 0 : 8 * 128])
            nc.scalar.dma_start(out=xb0b[:], in_=xt_d[0, :, 8 * 128 :])
            load_w(nc.gpsimd, 2)
            load_w(nc.sync, 3)
            load_w(nc.scalar, 4)
            load_w(nc.gpsimd, 5)
            load_w(nc.sync, 6)
            load_w(nc.scalar, 7)
            load_x(nc.gpsimd, 1)
            load_x(nc.scalar, 2)
            load_x(nc.sync, 3)
            load_x(nc.gpsimd, 4)
            load_x(nc.scalar, 5)
            load_x(nc.gpsimd, 6)
            load_x(nc.scalar, 7)

            # HAM warmup: dummies depend only on the memset tile, so
            # they run during the input-DMA window and lift the PE
            # clock gate to 8/8 before the first real matmul.
            ps_warm = ps_mm.tile([128, 2, 512], F32, tag="pAB", name="ps_warm")
            for i in range(NWARM):
                nc.tensor.matmul(
                    ps_warm[:, 0, 0:128], warm_f8, warm_f8,
                    start=True, stop=True, skip_group_check=True,
                )

            for b in range(NB):
                pAB = (
                    ps_warm if b == 0
                    else ps_mm.tile([128, 2, 512], F32, tag="pAB", name="pAB")
                )
                if 1 <= b <= 4:
                    # Bridge input-DMA jitter: a burst of dummies on
                    # this tile's (about-to-be-cleared) PSUM instance
                    # keeps the PE from idling into a HAM re-throttle
                    # while this b-tile's x block is still in flight.
                    for i in range(NFILL):
                        nc.tensor.matmul(
                            pAB[:, 0, 0:128], warm_f8, warm_f8,
                            start=True, stop=True, skip_group_check=True,
                        )
                for u in range(NU):
                    if u == 0:
                        # Bias pair first: ones x (16*bias, 0) — no x
                        # dependency, so the scheduler can hoist it
                        # into DMA-wait gaps.
                        lhsT = ones_f8[:]
                        w = wt_sb[NU - 1]
                    else:
                        lhsT = x_pair(b, u - 1)
                        w = wt_sb[u - 1]
                    nc.tensor.matmul(
                        pAB[:, 0, 0:NA], lhsT, w[:, :, 0:NA],
                        start=(u == 0), stop=(u == NU - 1), perf_mode=DR,
                        skip_group_check=True,
                    )
                    nc.tensor.matmul(
                        pAB[:, 1, 0:NBC], lhsT, w[:, :, NA : NA + NBC],
                        start=(u == 0), stop=(u == NU - 1), perf_mode=DR,
                        skip_group_check=True,
                    )

                # PSUM flat layout: cols 0..511 hazards, 512 base; the
                # relu'd base/pad cols 512..527 are junk the scan skips.
                haz = hazp.tile([128, TPW], BF16, tag="haz")
                nc.scalar.activation(
                    out=haz, in_=pAB[:, :, 0:NA],
                    func=mybir.ActivationFunctionType.Relu,
                )
                cum = outp.tile([128, T], BF16)
                nc.vector.tensor_tensor_scan(
                    out=cum,
                    data0=haz[:, 0:T],
                    data1=zeros,
                    initial=pAB[:, 1, T - NA : T - NA + 1],
                    op0=mybir.AluOpType.add,
                    op1=mybir.AluOpType.bypass,
                )
                if b >= NB - 2:
                    # Tail tiles: split the write across two rings so
                    # the final transfer halves.
                    nc.sync.dma_start(
                        out=out_d[128 * b : 128 * b + 64, :], in_=cum[0:64, :]
                    )
                    nc.scalar.dma_start(
                        out=out_d[128 * b + 64 : 128 * (b + 1), :],
                        in_=cum[64:128, :],
                    )
                else:
                    nc.sync.dma_start(
                        out=out_d[128 * b : 128 * (b + 1), :], in_=cum
                    )

    nc.compile()
    return nc


def _prep_in_maps(x, W_hazard, b_hazard, W_base, b_base):
    x = np.asarray(x, dtype=np.float32)
    W_aug = np.zeros((TPW, D), np.float32)
    W_aug[0:T] = np.asarray(W_hazard, np.float32)
    W_aug[T] = np.asarray(W_base, np.float32)[0]
    wt3 = np.zeros((128, NWS, TPW), NP_F8)
    wt3[:, 0:NKS, :] = (
        W_aug.T.astype(NP_F8).reshape(NKS, 128, TPW).transpose(1, 0, 2)
    )
    bias_row = np.zeros(TPW, np.float32)
    bias_row[0:T] = np.asarray(b_hazard, np.float32)
    bias_row[T] = np.asarray(b_base, np.float32)[0]
    wt3[0, NKS, :] = (bias_row / ONES).astype(NP_F8)
    wt3 = np.ascontiguousarray(wt3)

    x8 = x.astype(NP_F8)
    in_maps = []
    for i in range(NCORES):
        shard = x8[BLOC * i : BLOC * (i + 1)]  # [1024, 2048]
        # xt4[b, p, j*128+c] = shard[128b+c, 128j+p]
        xt4 = np.ascontiguousarray(
            shard.reshape(NB, 128, NKS, 128)  # [b, c, j, p]
            .transpose(0, 3, 2, 1)            # [b, p, j, c]
            .reshape(NB, 128, NKS * 128)
        )
        in_maps.append({"xt": xt4, "wt": wt3})
    return in_maps


_NC_CACHE = None


def kernel(x, W_hazard, b_hazard, W_base, b_base):
    global _NC_CACHE
    if _NC_CACHE is None:
        _NC_CACHE = _build_program()
    nc = _NC_CACHE

    in_maps = _prep_in_maps(x, W_hazard, b_hazard, W_base, b_base)
    res = run_bass_kernel_spmd(nc, in_maps, list(range(NCORES)))
    return np.concatenate(
        [res.results[i]["out"].astype(np.float32) for i in range(NCORES)], axis=0
    )


# revision 29
# speedup vs baseline: 1.1477x; 1.0058x over previous
"""Cumulative-probability head on 8 Trainium2 NeuronCores.

out[b, j] = sum_{i<=j} relu(x @ W_h^T + b_h)[b, i] + (x @ W_base^T + b_base)[b]

Data-parallel: x is sharded along batch (1024 rows per core); the small
weights are replicated. Inputs are cast to fp8 e4m3 on the host (the
2e-2 rel-err budget dwarfs the ~4e-3 this costs) so the matmul runs in
DoubleRow mode: 2 fp8 weights per PE cell, K=256 contracted per pass —
half the PE stream cycles of fp32r, and a quarter of the input DMA
bytes. Per core:

  1. xt4: per-batch-tile blocks [128, 16, 128] fp8 (contiguous 256 KB
     DMA each). wt3: [128, 18, 528] fp8 (528 = 512 hazard cols + base
     col + 15 pad, keeping the DoubleRow k-pair stride 1056 B a
     multiple of 16); subtile 16 carries 16*bias in partition 0,
     subtile 17 is zero. A memset tile of 0.0625 (exact in e4m3)
     supplies the ones operand: k-pair 8 = ones x (16*bias, 0)
     contributes exactly the bias row, folding the bias add into the
     matmul. DMA FIFO order per ring is need-order: all w chunks ahead
     of x blocks 1..7, since b-tile 0 consumes one w chunk per ~240 ns
     and its completion gates the whole epilogue pipeline.
  2. The PE HAM clock-gate needs ~3.4 us of sustained busy to lift the
     1.2 GHz cold throttle, and a mid-kernel idle gap >3.4 us drops it
     back. ~30 dummy matmuls on a zeroed tile burn the warmup during
     the input-DMA window; each b-tile's group opens with the bias
     pair (no x dependency, schedulable early) plus a few dummy
     matmuls on the next PSUM instance to bridge input-DMA jitter
     without going idle.
  3. Per b-tile: 9 DoubleRow accumulation steps into one two-bank PSUM
     tile [128, 2, 512] (cols 0..263 of each bank); bufs=4 -> 4
     b-tiles in flight across all 8 banks.
  4. Epilogue per b-tile: one ScalarE ReLU over a strided 2-bank PSUM
     read (bf16 out), then the DVE cumsum (tensor_tensor_scan) seeded
     with the base-hazard column read straight from PSUM. Output DMA
     in bf16 on the Sync ring (ScalarE stays on ReLUs).
"""

import ml_dtypes
import numpy as np

import concourse.bass as bass
import concourse.tile as tile
from concourse import bacc, mybir
from concourse.bass_utils import run_bass_kernel_spmd

B, D, T = 8192, 2048, 512
NCORES = 8
BLOC = B // NCORES            # 1024 rows per core
NB = BLOC // 128              # 8 batch tiles per core
NKS = D // 128                # 16 data contraction subtiles
NWS = NKS + 2                 # +bias +zeros subtiles in w
NU = NKS // 2 + 1             # 9 DoubleRow k-pairs (bias + 8 data)
TPW = 528                     # padded output width (512 hazard + base + 15 pad)
NA = TPW // 2                 # 264 cols per PSUM bank
NBC = 250                     # bank-B streamed cols (248 hazards + base + 1)
NWARM = 26                    # HAM warmup matmuls
NFILL = 2                     # stall-filler dummies inside b-tile 0's k-loop
ONES = 0.0625                 # exact in e4m3; bias row is scaled by 1/ONES

F32 = mybir.dt.float32
BF16 = mybir.dt.bfloat16
F8 = mybir.dt.float8e4
DR = mybir.MatmulPerfMode.DoubleRow
NP_F8 = ml_dtypes.float8_e4m3


def _build_program():
    nc = bacc.Bacc("TRN2", target_bir_lowering=False, debug=False)

    xt_d = nc.dram_tensor("xt", [NB, 128, NKS * 128], F8, kind="ExternalInput")
    wt_d = nc.dram_tensor("wt", [128, NWS, TPW], F8, kind="ExternalInput")
    out_d = nc.dram_tensor("out", [BLOC, T], BF16, kind="ExternalOutput")

    with tile.TileContext(nc) as tc:
        with (
            tc.tile_pool(name="consts", bufs=1) as consts,
            tc.tile_pool(name="wt", bufs=1) as wtp,
            tc.tile_pool(name="xt", bufs=1) as xtp,
            tc.tile_pool(name="haz", bufs=6) as hazp,
            tc.tile_pool(name="outp", bufs=6) as outp,
            tc.tile_pool(name="ps_mm", bufs=4, space="PSUM") as ps_mm,
        ):
            zeros = consts.tile([128, T], BF16)
            nc.vector.memset(zeros, 0.0)
            warm_f8 = consts.tile([128, 128], F8)
            nc.vector.memset(warm_f8, 0.0)
            ones_f8 = consts.tile([128, 2, 128], F8)
            nc.vector.memset(ones_f8, ONES)

            wt_sb = [
                wtp.tile([128, 2, TPW], F8, tag=f"wt{u}", name=f"wt{u}")
                for u in range(NU)
            ]

            xb0a = xtp.tile([128, 8, 128], F8, tag="xb0a")
            xb0b = xtp.tile([128, 8, 128], F8, tag="xb0b")
            xb = [
                xtp.tile([128, NKS, 128], F8, tag=f"xb{b}", name=f"xb{b}")
                for b in range(1, NB)
            ]

            def load_w(ring, u):
                ring.dma_start(
                    out=wt_sb[u][:], in_=wt_d[:, 2 * u : 2 * u + 2, :]
                )

            def load_x(ring, b):
                ring.dma_start(out=xb[b - 1][:], in_=xt_d[b, :, :])

            def x_pair(b, u):
                j0 = 2 * u
                if b == 0:
                    return (
                        xb0a[:, j0 : j0 + 2, :]
                        if j0 < 8
                        else xb0b[:, j0 - 8 : j0 - 6, :]
                    )
                return xb[b - 1][:, j0 : j0 + 2, :]

            # Need-ordered FIFO per ring: w pair 0 and both halves of x
            # block 0 gate the first matmuls; all remaining w chunks
            # (which gate b-tile 0's completion) ahead of x blocks.
            # w8 (the bias chunk) leads its ring as a plain dependency-
            # free 132 KB transfer: the bias pair opens every
            # accumulation group, so it gates every tile, and giving
            # it any producer dependency (e.g. a memset of its zero
            # half) makes the scheduler demote its issue slot.
            load_w(nc.gpsimd, 8)
            load_w(nc.sync, 0)
            load_w(nc.scalar, 1)
            nc.sync.dma_start(out=xb0a[:], in_=xt_d[0, :, 0 : 8 * 128])
            nc.scalar.dma_start(out=xb0b[:], in_=xt_d[0, :, 8 * 128 :])
            load_w(nc.gpsimd, 2)
            load_w(nc.sync, 3)
            load_w(nc.scalar, 4)
            load_w(nc.gpsimd, 5)
            load_w(nc.sync, 6)
            load_w(nc.scalar, 7)
            load_x(nc.gpsimd, 1)
            load_x(nc.scalar, 2)
            load_x(nc.sync, 3)
            load_x(nc.gpsimd, 4)
            load_x(nc.scalar, 5)
            load_x(nc.gpsimd, 6)
            load_x(nc.scalar, 7)

            # HAM warmup: dummies depend only on the memset tile, so
            # they run during the input-DMA window and lift the PE
            # clock gate to 8/8 before the first real matmul.
            ps_warm = ps_mm.tile([128, 2, 512], F32, tag="pAB", name="ps_warm")
            for i in range(NWARM):
                nc.tensor.matmul(
                    ps_warm[:, 0, 0:128], warm_f8, warm_f8,
                    start=True, stop=True, skip_group_check=True,
                )

            pAB_next = ps_mm.tile([128, 2, 512], F32, tag="pAB", name="pAB1")
            for b in range(NB):
                if b == 0:
                    pAB = ps_warm
                else:
                    pAB = pAB_next
                    if b < NB - 1:
                        pAB_next = ps_mm.tile(
                            [128, 2, 512], F32, tag="pAB", name="pAB"
                        )
                for u in range(NU):
                    if u == 0:
                        # Bias pair first: ones x (16*bias, 0) — no x
                        # dependency, so the scheduler can hoist it
                        # into DMA-wait gaps.
                        lhsT = ones_f8[:]
                        w = wt_sb[NU - 1]
                    else:
                        lhsT = x_pair(b, u - 1)
                        w = wt_sb[u - 1]
                    nc.tensor.matmul(
                        pAB[:, 0, 0:NA], lhsT, w[:, :, 0:NA],
                        start=(u == 0), stop=(u == NU - 1), perf_mode=DR,
                        skip_group_check=True,
                    )
                    nc.tensor.matmul(
                        pAB[:, 1, 0:NBC], lhsT, w[:, :, NA : NA + NBC],
                        start=(u == 0), stop=(u == NU - 1), perf_mode=DR,
                        skip_group_check=True,
                    )
                    if b == 0 and u < NU - 1:
                        # b-tile 0 is paced by w-chunk arrival; filler
                        # dummies on the NEXT tile's (about-to-be-
                        # cleared) PSUM instance turn the micro-stalls
                        # into PE activity so the HAM window stays busy
                        # and the clock gate lifts on schedule.
                        for i in range(NFILL):
                            nc.tensor.matmul(
                                pAB_next[:, 0, 0:128], warm_f8, warm_f8,
                                start=True, stop=True, skip_group_check=True,
                            )

                # PSUM flat layout: cols 0..511 hazards, 512 base; the
                # relu'd base/pad cols 512..527 are junk the scan skips.
                haz = hazp.tile([128, TPW], BF16, tag="haz")
                nc.scalar.activation(
                    out=haz, in_=pAB[:, :, 0:NA],
                    func=mybir.ActivationFunctionType.Relu,
                )
                cum = outp.tile([128, T], BF16)
                nc.vector.tensor_tensor_scan(
                    out=cum,
                    data0=haz[:, 0:T],
                    data1=zeros,
                    initial=pAB[:, 1, T - NA : T - NA + 1],
                    op0=mybir.AluOpType.add,
                    op1=mybir.AluOpType.bypass,
                )
                if b >= NB - 2:
                    # Tail tiles: split the write across two rings so
                    # the final transfer halves.
                    nc.sync.dma_start(
                        out=out_d[128 * b : 128 * b + 64, :], in_=cum[0:64, :]
                    )
                    nc.scalar.dma_start(
                        out=out_d[128 * b + 64 : 128 * (b + 1), :],
                        in_=cum[64:128, :],
                    )
                else:
                    nc.sync.dma_start(
                        out=out_d[128 * b : 128 * (b + 1), :], in_=cum
                    )

    nc.compile()
    return nc


def _prep_in_maps(x, W_hazard, b_hazard, W_base, b_base):
    x = np.asarray(x, dtype=np.float32)
    W_aug = np.zeros((TPW, D), np.float32)
    W_aug[0:T] = np.asarray(W_hazard, np.float32)
    W_aug[T] = np.asarray(W_base, np.float32)[0]
    wt3 = np.zeros((128, NWS, TPW), NP_F8)
    wt3[:, 0:NKS, :] = (
        W_aug.T.astype(NP_F8).reshape(NKS, 128, TPW).transpose(1, 0, 2)
    )
    bias_row = np.zeros(TPW, np.float32)
    bias_row[0:T] = np.asarray(b_hazard, np.float32)
    bias_row[T] = np.asarray(b_base, np.float32)[0]
    wt3[0, NKS, :] = (bias_row / ONES).astype(NP_F8)
    wt3 = np.ascontiguousarray(wt3)

    x8 = x.astype(NP_F8)
    in_maps = []
    for i in range(NCORES):
        shard = x8[BLOC * i : BLOC * (i + 1)]  # [1024, 2048]
        # xt4[b, p, j*128+c] = shard[128b+c, 128j+p]
        xt4 = np.ascontiguousarray(
            shard.reshape(NB, 128, NKS, 128)  # [b, c, j, p]
            .transpose(0, 3, 2, 1)            # [b, p, j, c]
            .reshape(NB, 128, NKS * 128)
        )
        in_maps.append({"xt": xt4, "wt": wt3})
    return in_maps


_NC_CACHE = None


def kernel(x, W_hazard, b_hazard, W_base, b_base):
    global _NC_CACHE
    if _NC_CACHE is None:
        _NC_CACHE = _build_program()
    nc = _NC_CACHE

    in_maps = _prep_in_maps(x, W_hazard, b_hazard, W_base, b_base)
    res = run_bass_kernel_spmd(nc, in_maps, list(range(NCORES)))
    return np.concatenate(
        [res.results[i]["out"].astype(np.float32) for i in range(NCORES)], axis=0
    )
